# revision 2
# baseline (speedup 1.0000x reference)
"""Trainium2 Bass kernel for nn_FusedKQnA (sparse attention with learned
queries + depthwise stride-2 conv aggregation).

Math restructuring (vs the reference):
  - k is never materialized: qkT = x^T @ (Wk @ QW) with QW the block-diagonal
    arrangement of the scaled learned queries -> one (128->32) matmul.
  - The global max subtractions inside the two exp() calls cancel exactly
    between numerator and denominator, so they are dropped.
  - The 1024-channel depthwise conv never materializes.  With
    r = 1/sum_den (computed as exp(-ln(den)), same ACT table set) define
        gamma[t,h,ij] = sum_q kern[t,q*8+h] * r[q*8+h,ij] * cost[n_t(ij),q*8+h]
    Then out_pre[(h,c),ij] = sum_t gamma[t,h,ij] * v[n_t(ij),(h,c)]  (256 ch)
    and out = Wout @ out_pre.
  - gamma's q-contraction + broadcast over the 32 channels of each head is a
    single small PE matmul per (tap, channel-chunk) with a one-hot*kern
    stationary operand.

Performance structure (v2):
  - x is host-permuted into parity order (rows [0,2..54,1,3..55], cols
    [1,3..55,0,2..54]) so the stride-2 tap views of the cost/v planes become
    stride-1 slices of parity-region planes.  All rc multiplies then run in
    the DVE's fast packed mode.
  - cost plane duplicates the odd-col region so all 9 tap views start at
    even (4B-aligned) offsets.
  - PE warm-up: dummy matmuls issued during the initial x DMA keep the
    PE HAM activity monitor busy so the real matmuls run at 2.4 GHz
    instead of the cold 1.2 GHz.
  - All small weights are pre-transposed/pre-cast to bf16 on the host so
    every constant load is a plain DMA (no gpsimd conversion ops).
  - Tap-sum add tree runs on GpSimd (SBUF-only), products on DVE, plane
    drains on the Scalar engine: roughly balanced engine load.

Sharding: pure data parallel over batch: 16 batches -> 8 cores x 2.
"""

import os
from contextlib import ExitStack

import numpy as np

import concourse.bass as bass
import concourse.mybir as mybir
import concourse.tile as tile
from concourse import bacc
from concourse.bass_utils import run_bass_kernel_spmd

# Problem constants (hardcoded per spec nn_FusedKQnA_1726576854813)
N_Q, N_HEADS, KSIZE, STRIDE, PADDING = 4, 4, 3, 2, 1
B, C, H, W = 16, 128, 56, 56
HC = C // N_HEADS            # 32 head channels
HP = N_HEADS * STRIDE        # 8 effective heads
CS = C * STRIDE              # 256
G = N_Q * HP                 # 32 kernel groups
HO, WO = H // STRIDE, W // STRIDE   # 28, 28
NCORES = 8
BPC = B // NCORES            # batches per core

TAPS = [(di, dj) for di in (-1, 0, 1) for dj in (-1, 0, 1)]
N_STRIPS = 2                 # output rows split into strips of 14 (392 px)
RPS = HO // N_STRIPS         # 14 rows per strip

# plane geometry: rows [even 0:28 | odd-slots 28:57], 57 rows
#   row start per di: 0 -> 0, -1 -> 28, +1 -> 29   (odd slot 28 is row -1 = 0)
PROWS = 57
ROW0 = {0: 0, -1: 28, 1: 29}
# cost plane cols [O1 0:28 | E 28:56 | pad 56:58 | O2 58:86]
#   O1 slot k = col 2k-1 (slot 0 zero), E slot 28+j = col 2j,
#   O2 slot 58+j = col 2j+1.  All tap-view col starts even (aligned).
CCOLS = 86
CCOL0 = {-1: 0, 0: 28, 1: 58}
# v plane cols [Z 0 | O 1:29 | E 29:57]; O slot 1+m = col 2m+1
VCOLS = 57
VCOL0 = {-1: 0, 1: 1, 0: 29}

N_RT = 8                     # row tiles per batch (7 natural rows each)
RT = H // N_RT               # 7

F32 = mybir.dt.float32
BF16 = mybir.dt.bfloat16

_BUILD_CACHE = {}


def _host_weights(Wk, Wv, Wout, q_param, attn_scale, rpb_table):
    """Precompute all small weight tensors on the host (bf16, pre-laid-out)."""
    import ml_dtypes
    q = q_param.reshape(N_Q, HP, HC).astype(np.float64) * (HC ** -0.5)
    QW = np.zeros((CS, G), np.float64)
    for qi in range(N_Q):
        for h in range(HP):
            QW[h * HC:(h + 1) * HC, qi * HP + h] = q[qi, h]
    wkq = (Wk.astype(np.float64) @ QW).astype(np.float32)        # (128, 32)

    rpb_exp = np.exp(rpb_table.astype(np.float64))               # (9, 32)
    kern_num = (rpb_exp * attn_scale.astype(np.float64))         # (9, 32)

    # denominator conv kernels as diagonal matmul weights, pre-transposed to
    # [g, tap, m] so the SBUF tile loads with a plain DMA
    denk = np.zeros((G, KSIZE * KSIZE, G), np.float32)
    for t in range(KSIZE * KSIZE):
        for g in range(G):
            denk[g, t, g] = rpb_exp[t, g]

    # gamma-broadcast stationary operands, stacked 3 taps per row-group for
    # tile_position packing, pre-transposed to [3*G, grp, ch, 128]
    kmat = np.zeros((3 * G, 3, 2, 128), np.float32)
    for t in range(KSIZE * KSIZE):
        grp, tau = divmod(t, 3)
        for ch in range(2):
            for g in range(G):
                h = g % HP
                if h // 4 == ch:
                    m0 = (h % 4) * HC
                    kmat[tau * G + g, grp, ch, m0:m0 + HC] = kern_num[t, g]

    # woutT pre-arranged to [128, kc, 256]
    woutT = np.ascontiguousarray(
        Wout.T.astype(np.float32).reshape(2, 128, CS).transpose(1, 0, 2))

    return dict(wkq=wkq.astype(ml_dtypes.bfloat16),
                denk=denk.astype(ml_dtypes.bfloat16),
                kmat=kmat.astype(ml_dtypes.bfloat16),
                woutT=woutT.astype(ml_dtypes.bfloat16),
                wv=np.ascontiguousarray(Wv.astype(ml_dtypes.bfloat16)))


def _build_program():
    nc = bacc.Bacc("TRN2", target_bir_lowering=False, debug=False,
                   enable_asserts=False, num_devices=NCORES)

    # x arrives host-permuted: rows [0,2..54 | 1,3..55], cols [1,3..55 | 0,2..54]
    xe_d = [nc.dram_tensor(f"xe{b}", [C, H // 2, W], BF16,
                           kind="ExternalInput").ap() for b in range(BPC)]
    xo_d = [nc.dram_tensor(f"xo{b}", [C, H // 2, W], BF16,
                           kind="ExternalInput").ap() for b in range(BPC)]
    wkq_d = nc.dram_tensor("wkq", [C, G], BF16, kind="ExternalInput").ap()
    wv_d = nc.dram_tensor("wv", [C, CS], BF16, kind="ExternalInput").ap()
    denk_d = nc.dram_tensor("denk", [G, 9, G], BF16, kind="ExternalInput").ap()
    kmat_d = nc.dram_tensor("kmat", [3 * G, 3, 2, 128], BF16,
                            kind="ExternalInput").ap()
    woutT_d = nc.dram_tensor("woutT", [128, 2, CS], BF16,
                             kind="ExternalInput").ap()
    out_d = nc.dram_tensor("out", [BPC, CS, HO, WO], F32,
                           kind="ExternalOutput").ap()

    with tile.TileContext(nc) as tc, ExitStack() as ctx:
        _kernel_body(ctx, tc, out_d, xe_d, xo_d, wkq_d, wv_d, denk_d, kmat_d,
                     woutT_d)

    _pin_act_tables()
    nc.compile()
    return nc


def _pin_act_tables():
    """Force one ACT table set (natural_log_exp_and_others) for Exp+Ln so the
    scheduler doesn't thrash table loads between them."""
    from concourse import hw_specs
    import concourse.bacc as bacc_mod
    if getattr(bacc_mod, "_act_tables_pinned", False):
        return
    orig = hw_specs.get_activation_tables

    def patched(arch):
        tabs = dict(orig(arch))
        keep = "natural_log_exp_and_others"
        for name in list(tabs):
            if name == keep:
                continue
            fns = tabs[name]
            if any(str(f).endswith((".Exp", ".Ln")) for f in fns):
                tabs[name] = type(fns)()
        return tabs

    bacc_mod.get_activation_tables = patched
    bacc_mod._act_tables_pinned = True


def _kernel_body(ctx, tc, out_d, xe_d, xo_d, wkq_d, wv_d, denk_d, kmat_d,
                 woutT_d):
    nc = tc.nc

    consts = ctx.enter_context(tc.tile_pool(name="consts", bufs=1))
    planes = ctx.enter_context(tc.tile_pool(name="planes", bufs=1))
    xpool = ctx.enter_context(tc.tile_pool(name="xpool", bufs=1))
    small = ctx.enter_context(tc.tile_pool(name="small", bufs=2))
    rcpool = ctx.enter_context(tc.tile_pool(name="rcpool", bufs=2))
    prod_pool = ctx.enter_context(tc.tile_pool(name="prod", bufs=4))
    opre_pool = ctx.enter_context(tc.tile_pool(name="opre", bufs=2))
    outs_pool = ctx.enter_context(tc.tile_pool(name="outs", bufs=4))

    ps = ctx.enter_context(tc.tile_pool(name="ps", bufs=2, space="PSUM"))

    # ---- PE warm-up fodder: zero tile, no external deps ----
    wz = consts.tile([128, 512], BF16)
    nc.gpsimd.memset(wz, 0.0)
    for i in range(14):
        warm_ps = ps.tile([128, 512], F32, tag="mm", bufs=2, name="warm_ps")
        nc.tensor.matmul(warm_ps, wz[:, 0:128], wz, start=True, stop=True)

    # ---- constants into SBUF (plain DMAs, host-prepared layouts) ----
    wkq_sb = consts.tile([C, G], BF16)
    nc.sync.dma_start(out=wkq_sb, in_=wkq_d)
    wv_sb = consts.tile([C, CS], BF16)
    nc.sync.dma_start(out=wv_sb, in_=wv_d)
    denk_sb = consts.tile([G, 9, G], BF16)
    nc.sync.dma_start(out=denk_sb, in_=denk_d)
    kmat_sb = consts.tile([3 * G, 3, 2, 128], BF16)
    nc.sync.dma_start(out=kmat_sb, in_=kmat_d)
    woutT_sb = consts.tile([128, 2, CS], BF16)
    nc.sync.dma_start(out=woutT_sb, in_=woutT_d)

    # ---- x into SBUF, split even/odd row chunks for earlier compute ----
    x_sb = {}
    for b in range(BPC):
        x_sb[(b, 0)] = xpool.tile([C, H // 2, W], BF16, name=f"xe{b}")
        nc.sync.dma_start(out=x_sb[(b, 0)], in_=xe_d[b])
        x_sb[(b, 1)] = xpool.tile([C, H // 2, W], BF16, name=f"xo{b}")
        nc.sync.dma_start(out=x_sb[(b, 1)], in_=xo_d[b])

    # ---- persistent parity planes (zero pads set once) ----
    cost_pl = [planes.tile([G, PROWS, CCOLS], BF16, tag=f"cost{b}",
                           name=f"cost_pl{b}") for b in range(BPC)]
    v_pl = [[planes.tile([128, PROWS, VCOLS], BF16, tag=f"v{b}_{chn}",
                         name=f"v_pl{b}_{chn}") for chn in range(2)]
            for b in range(BPC)]
    for pl in cost_pl:
        nc.gpsimd.memset(pl[:, 28, :], 0.0)    # row -1
        nc.gpsimd.memset(pl[:, :, 0], 0.0)     # col -1 (O1 slot 0)
    for b in range(BPC):
        for chn in range(2):
            pl = v_pl[b][chn]
            nc.gpsimd.memset(pl[:, 28, :], 0.0)
            nc.gpsimd.memset(pl[:, :, 0], 0.0)

    def cview(b, di, dj):
        r0, c0 = ROW0[di], CCOL0[dj]
        return cost_pl[b][:, r0:r0 + 28, c0:c0 + 28]

    def vview(b, chn, di, dj):
        r0, c0 = ROW0[di], VCOL0[dj]
        return v_pl[b][chn][:, r0:r0 + 28, c0:c0 + 28]

    def dst_rows(rt):
        # row tile rt covers permuted rows 7rt..7rt+6; even tiles (rt<4) map
        # to plane rows 7rt.., odd tiles to plane rows 29+7(rt-4)..
        return 7 * rt if rt < 4 else 29 + 7 * (rt - 4)

    # ---- phase A1: qk matmuls + exp into cost planes ----
    for b in range(BPC):
        for rt in range(N_RT):
            half, idx = (0, rt) if rt < 4 else (1, rt - 4)
            qk_ps = ps.tile([G, RT, W], F32, tag="mm", bufs=2, name="qk_ps")
            nc.tensor.matmul(qk_ps, wkq_sb,
                             x_sb[(b, half)][:, idx * RT:(idx + 1) * RT, :],
                             start=True, stop=True)
            rd = dst_rows(rt)
            pl = cost_pl[b]
            # src cols: [0:28] odd (1,3..55), [28:56] even (0,2..54)
            nc.scalar.activation(out=pl[:, rd:rd + RT, 1:28],
                                 in_=qk_ps[:, :, 0:27],
                                 func=mybir.ActivationFunctionType.Exp)
            nc.scalar.activation(out=pl[:, rd:rd + RT, 28:56],
                                 in_=qk_ps[:, :, 28:56],
                                 func=mybir.ActivationFunctionType.Exp)
            nc.scalar.activation(out=pl[:, rd:rd + RT, 58:86],
                                 in_=qk_ps[:, :, 0:28],
                                 func=mybir.ActivationFunctionType.Exp)

    # ---- phase B: den conv as diagonal matmuls on PE + r = exp(-ln(den)) ----
    r_sb = {}
    for b in range(BPC):
        den_ps = ps.tile([G, 2, 512], F32, tag="gam", bufs=3, name="den_ps")
        for s in range(N_STRIPS):
            dv = den_ps[:, s, :RPS * WO].rearrange("p (a c) -> p a c", a=RPS)
            for t, (di, dj) in enumerate(TAPS):
                nc.tensor.matmul(
                    dv, denk_sb[:, t, :],
                    cview(b, di, dj)[:, s * RPS:(s + 1) * RPS, :],
                    start=(t == 0), stop=(t == 8))
        lden = small.tile([G, 2, RPS, WO], F32, tag="lden", name="lden")
        dfull = den_ps[:, :, :RPS * WO].rearrange("p s (a c) -> p s a c",
                                                  a=RPS)
        nc.scalar.activation(out=lden, in_=dfull,
                             func=mybir.ActivationFunctionType.Ln)
        r_sb[b] = small.tile([G, 2, RPS, WO], BF16, tag="rr", name="rr")
        nc.scalar.activation(out=r_sb[b], in_=lden, scale=-1.0,
                             func=mybir.ActivationFunctionType.Exp)

    # ---- phase A2: v matmuls -> v planes ----
    for b in range(BPC):
        for chn in range(2):
            for rt in range(N_RT):
                half, idx = (0, rt) if rt < 4 else (1, rt - 4)
                v_ps = ps.tile([128, RT, W], F32, tag="mm", bufs=2,
                               name="v_ps")
                nc.tensor.matmul(v_ps, wv_sb[:, chn * 128:(chn + 1) * 128],
                                 x_sb[(b, half)][:, idx * RT:(idx + 1) * RT, :],
                                 start=True, stop=True)
                rd = dst_rows(rt)
                # one drain: dst cols [1:57] = [O slots 1:29 | E 29:57],
                # src [0:28]=odd, [28:56]=even
                dst = v_pl[b][chn][:, rd:rd + RT, 1:57].rearrange(
                    "p r (g c) -> p r g c", g=2)
                src = v_ps.rearrange("p r (g c) -> p r g c", g=2)
                nc.scalar.copy(out=dst, in_=src)

    # ---- phase C: rc[t] = cost_t * r (DVE, packed 4x mode) ----
    rc_st = {}
    for b in range(BPC):
        rc_st[b] = [rcpool.tile([3 * G, 2, RPS, WO], BF16,
                                tag=f"rc{grp}", name=f"rc_st{grp}")
                    for grp in range(3)]
        for t, (di, dj) in enumerate(TAPS):
            grp, tau = divmod(t, 3)
            cvs = cview(b, di, dj).rearrange("p (s a) c -> p s a c", s=2)
            nc.vector.tensor_mul(rc_st[b][grp][tau * G:(tau + 1) * G],
                                 cvs, r_sb[b])

    # ---- phases D+E per batch: packed gamma matmuls, products on DVE,
    #      tap-sum tree on GpSimd ----
    opre_sb = {}
    gsums = {}
    for grp in range(3):
        for b in range(BPC):
            for chn in range(2):
                gams = [ps.tile([128, 2, 512], F32, tag="gam", bufs=3,
                                name=f"gam_ps{tau}") for tau in range(3)]
                for s in range(N_STRIPS):
                    for tau in range(3):
                        gv = gams[tau][:, s, :RPS * WO].rearrange(
                            "p (a c) -> p a c", a=RPS)
                        nc.tensor.matmul(
                            gv, kmat_sb[tau * G:(tau + 1) * G, grp, chn, :],
                            rc_st[b][grp][tau * G:(tau + 1) * G, s],
                            start=True, stop=True,
                            tile_position=(tau * G, 0))
                ps_taps = []
                for tau in range(3):
                    t = grp * 3 + tau
                    di, dj = TAPS[t]
                    p_sb = prod_pool.tile([128, 2, RPS, WO], BF16,
                                          tag="p", bufs=9, name=f"p{tau}")
                    gfull = gams[tau][:, :, :RPS * WO].rearrange(
                        "p s (a c) -> p s a c", a=RPS)
                    vv = vview(b, chn, di, dj).rearrange(
                        "p (s a) c -> p s a c", s=2)
                    nc.vector.tensor_mul(p_sb, gfull, vv)
                    ps_taps.append(p_sb)
                gs = prod_pool.tile([128, 2 * RPS * WO], BF16,
                                    tag=f"gs{grp}_{b}_{chn}", bufs=1,
                                    name=f"gs{grp}{b}{chn}")
                flat = [p.rearrange("p s a c -> p (s a c)") for p in ps_taps]
                nc.gpsimd.tensor_add(gs, flat[0], flat[1])
                nc.gpsimd.tensor_add(gs, gs, flat[2])
                gsums[(grp, b, chn)] = gs
    for b in range(BPC):
        for chn in range(2):
            o_sb = opre_pool.tile([128, 2, RPS, WO], BF16,
                                  tag=f"opre{chn}", name=f"opre{chn}")
            of = o_sb.rearrange("p s a c -> p (s a c)")
            nc.gpsimd.tensor_add(of, gsums[(0, b, chn)], gsums[(1, b, chn)])
            nc.gpsimd.tensor_add(of, of, gsums[(2, b, chn)])
            for s in range(N_STRIPS):
                opre_sb[(b, chn, s)] = o_sb[:, s]

        for mo in range(2):
            for s in range(N_STRIPS):
                out_ps = ps.tile([128, RPS, WO], F32, tag="mm",
                                 bufs=2, name="out_ps")
                for kc in range(2):
                    nc.tensor.matmul(out_ps,
                                     woutT_sb[:, kc, mo * 128:(mo + 1) * 128],
                                     opre_sb[(b, kc, s)],
                                     start=(kc == 0), stop=(kc == 1))
                o_final = outs_pool.tile([128, RPS, WO], F32)
                nc.scalar.copy(out=o_final, in_=out_ps)
                nc.sync.dma_start(
                    out=out_d[b, mo * 128:(mo + 1) * 128,
                              s * RPS:(s + 1) * RPS, :],
                    in_=o_final)


def _install_ntff_shim():
    """bass_utils expects antenv.axon_hooks (absent in this checkout); shim it
    with the ctypes NTFF hook from trn_agent_boot so trace=True works."""
    import sys
    import types
    try:
        from antenv.axon_hooks import get_axon_ntff_profile_hook  # noqa: F401
        return
    except ImportError:
        pass
    try:
        from trn_agent_boot.trn_boot import _ntff_profile_via_ctypes
        hook = _ntff_profile_via_ctypes("/opt/axon/libaxon_pjrt.so")
    except Exception:
        hook = None
    mod = types.ModuleType("antenv.axon_hooks")
    mod._hook = hook
    mod.get_axon_ntff_profile_hook = lambda: mod._hook
    mod.set_axon_ntff_profile_hook = lambda h: setattr(mod, "_hook", h)
    sys.modules["antenv.axon_hooks"] = mod


def _get_program():
    if "nc" not in _BUILD_CACHE:
        _BUILD_CACHE["nc"] = _build_program()
    return _BUILD_CACHE["nc"]


# host-side pixel permutation: rows even-first, cols odd-first
_ROW_PERM = np.concatenate([np.arange(0, H, 2), np.arange(1, H, 2)])
_COL_PERM = np.concatenate([np.arange(1, W, 2), np.arange(0, W, 2)])


def kernel(x, Wk, Wv, Wout, q_param, attn_scale, rpb_table):
    import ml_dtypes
    x = np.asarray(x, dtype=np.float32)
    xp = x[:, :, _ROW_PERM][:, :, :, _COL_PERM].astype(ml_dtypes.bfloat16)
    wts = _host_weights(np.asarray(Wk), np.asarray(Wv), np.asarray(Wout),
                        np.asarray(q_param), np.asarray(attn_scale),
                        np.asarray(rpb_table))
    nc = _get_program()

    in_maps = []
    for c in range(NCORES):
        m = dict(wts)
        for b in range(BPC):
            xb = xp[c * BPC + b]
            m[f"xe{b}"] = np.ascontiguousarray(xb[:, :H // 2])
            m[f"xo{b}"] = np.ascontiguousarray(xb[:, H // 2:])
        in_maps.append(m)

    trace = bool(int(os.environ.get("KERNEL_TRACE", "0")))
    if trace:
        _install_ntff_shim()
    res = run_bass_kernel_spmd(nc, in_maps, core_ids=list(range(NCORES)),
                               trace=trace)
    _BUILD_CACHE["last_results"] = res

    out = np.empty((B, CS, HO, WO), np.float32)
    for c in range(NCORES):
        out[c * BPC:(c + 1) * BPC] = res.results[c]["out"]
    return out


# revision 3
# speedup vs baseline: 1.1488x; 1.1488x over previous
"""Trainium2 Bass kernel for nn_FusedKQnA (sparse attention with learned
queries + depthwise stride-2 conv aggregation).

Math restructuring (vs the reference):
  - k is never materialized: qkT = x^T @ (Wk @ QW) with QW the block-diagonal
    arrangement of the scaled learned queries -> one (128->32) matmul.
  - The global max subtractions inside the two exp() calls cancel exactly
    between numerator and denominator, so they are dropped.
  - The 1024-channel depthwise conv never materializes.  With
    r = 1/sum_den (computed as exp(-ln(den)), same ACT table set) define
        gamma[t,h,ij] = sum_q kern[t,q*8+h] * r[q*8+h,ij] * cost[n_t(ij),q*8+h]
    Then out_pre[(h,c),ij] = sum_t gamma[t,h,ij] * v[n_t(ij),(h,c)]  (256 ch)
    and out = Wout @ out_pre.
  - gamma's q-contraction + broadcast over the 32 channels of each head is a
    single small PE matmul per (tap, channel-chunk) with a one-hot*kern
    stationary operand.

Performance structure (v2):
  - x is host-permuted into parity order (rows [0,2..54,1,3..55], cols
    [1,3..55,0,2..54]) so the stride-2 tap views of the cost/v planes become
    stride-1 slices of parity-region planes.  All rc multiplies then run in
    the DVE's fast packed mode.
  - cost plane duplicates the odd-col region so all 9 tap views start at
    even (4B-aligned) offsets.
  - PE warm-up: dummy matmuls issued during the initial x DMA keep the
    PE HAM activity monitor busy so the real matmuls run at 2.4 GHz
    instead of the cold 1.2 GHz.
  - All small weights are pre-transposed/pre-cast to bf16 on the host so
    every constant load is a plain DMA (no gpsimd conversion ops).
  - Tap-sum add tree runs on GpSimd (SBUF-only), products on DVE, plane
    drains on the Scalar engine: roughly balanced engine load.

Sharding: pure data parallel over batch: 16 batches -> 8 cores x 2.
"""

import os
from contextlib import ExitStack

import numpy as np

import concourse.bass as bass
import concourse.mybir as mybir
import concourse.tile as tile
from concourse import bacc
from concourse.bass_utils import run_bass_kernel_spmd

# Problem constants (hardcoded per spec nn_FusedKQnA_1726576854813)
N_Q, N_HEADS, KSIZE, STRIDE, PADDING = 4, 4, 3, 2, 1
B, C, H, W = 16, 128, 56, 56
HC = C // N_HEADS            # 32 head channels
HP = N_HEADS * STRIDE        # 8 effective heads
CS = C * STRIDE              # 256
G = N_Q * HP                 # 32 kernel groups
HO, WO = H // STRIDE, W // STRIDE   # 28, 28
NCORES = 8
BPC = B // NCORES            # batches per core

TAPS = [(di, dj) for di in (-1, 0, 1) for dj in (-1, 0, 1)]
N_STRIPS = 2                 # output rows split into strips of 14 (392 px)
RPS = HO // N_STRIPS         # 14 rows per strip

# plane geometry: rows [even 0:28 | odd-slots 28:57], 57 rows
#   row start per di: 0 -> 0, -1 -> 28, +1 -> 29   (odd slot 28 is row -1 = 0)
PROWS = 57
ROW0 = {0: 0, -1: 28, 1: 29}
# cost plane cols [O1 0:28 | E 28:56 | pad 56:58 | O2 58:86]
#   O1 slot k = col 2k-1 (slot 0 zero), E slot 28+j = col 2j,
#   O2 slot 58+j = col 2j+1.  All tap-view col starts even (aligned).
CCOLS = 86
CCOL0 = {-1: 0, 0: 28, 1: 58}
# v plane cols [Z 0 | O 1:29 | E 29:57]; O slot 1+m = col 2m+1
VCOLS = 57
VCOL0 = {-1: 0, 1: 1, 0: 29}

N_RT = 8                     # row tiles per batch (7 natural rows each)
RT = H // N_RT               # 7

F32 = mybir.dt.float32
BF16 = mybir.dt.bfloat16

_BUILD_CACHE = {}


def _host_weights(Wk, Wv, Wout, q_param, attn_scale, rpb_table):
    """Precompute all small weight tensors on the host (bf16, pre-laid-out)."""
    import ml_dtypes
    q = q_param.reshape(N_Q, HP, HC).astype(np.float64) * (HC ** -0.5)
    QW = np.zeros((CS, G), np.float64)
    for qi in range(N_Q):
        for h in range(HP):
            QW[h * HC:(h + 1) * HC, qi * HP + h] = q[qi, h]
    wkq = (Wk.astype(np.float64) @ QW).astype(np.float32)        # (128, 32)

    rpb_exp = np.exp(rpb_table.astype(np.float64))               # (9, 32)
    kern_num = (rpb_exp * attn_scale.astype(np.float64))         # (9, 32)

    # denominator conv kernels as diagonal matmul weights, pre-transposed to
    # [g, tap, m] so the SBUF tile loads with a plain DMA
    denk = np.zeros((G, KSIZE * KSIZE, G), np.float32)
    for t in range(KSIZE * KSIZE):
        for g in range(G):
            denk[g, t, g] = rpb_exp[t, g]

    # gamma-broadcast stationary operands, stacked 3 taps per row-group for
    # tile_position packing, pre-transposed to [3*G, grp, ch, 128]
    kmat = np.zeros((3 * G, 3, 2, 128), np.float32)
    for t in range(KSIZE * KSIZE):
        grp, tau = divmod(t, 3)
        for ch in range(2):
            for g in range(G):
                h = g % HP
                if h // 4 == ch:
                    m0 = (h % 4) * HC
                    kmat[tau * G + g, grp, ch, m0:m0 + HC] = kern_num[t, g]

    # woutT pre-arranged to [128, kc, 256]
    woutT = np.ascontiguousarray(
        Wout.T.astype(np.float32).reshape(2, 128, CS).transpose(1, 0, 2))

    return dict(wkq=wkq.astype(ml_dtypes.bfloat16),
                denk=denk.astype(ml_dtypes.bfloat16),
                kmat=kmat.astype(ml_dtypes.bfloat16),
                woutT=woutT.astype(ml_dtypes.bfloat16),
                wv=np.ascontiguousarray(Wv.astype(ml_dtypes.bfloat16)))


def _build_program():
    nc = bacc.Bacc("TRN2", target_bir_lowering=False, debug=False,
                   enable_asserts=False, num_devices=NCORES)

    # x arrives host-permuted: rows [0,2..54 | 1,3..55], cols [1,3..55 | 0,2..54]
    xe_d = [nc.dram_tensor(f"xe{b}", [C, H // 2, W], BF16,
                           kind="ExternalInput").ap() for b in range(BPC)]
    xo_d = [nc.dram_tensor(f"xo{b}", [C, H // 2, W], BF16,
                           kind="ExternalInput").ap() for b in range(BPC)]
    wkq_d = nc.dram_tensor("wkq", [C, G], BF16, kind="ExternalInput").ap()
    wv_d = nc.dram_tensor("wv", [C, CS], BF16, kind="ExternalInput").ap()
    denk_d = nc.dram_tensor("denk", [G, 9, G], BF16, kind="ExternalInput").ap()
    kmat_d = nc.dram_tensor("kmat", [3 * G, 3, 2, 128], BF16,
                            kind="ExternalInput").ap()
    woutT_d = nc.dram_tensor("woutT", [128, 2, CS], BF16,
                             kind="ExternalInput").ap()
    out_d = nc.dram_tensor("out", [BPC, CS, HO, WO], F32,
                           kind="ExternalOutput").ap()

    with tile.TileContext(nc) as tc, ExitStack() as ctx:
        _kernel_body(ctx, tc, out_d, xe_d, xo_d, wkq_d, wv_d, denk_d, kmat_d,
                     woutT_d)

    _pin_act_tables()
    nc.compile()
    return nc


def _pin_act_tables():
    """Force one ACT table set (natural_log_exp_and_others) for Exp+Ln so the
    scheduler doesn't thrash table loads between them."""
    from concourse import hw_specs
    import concourse.bacc as bacc_mod
    if getattr(bacc_mod, "_act_tables_pinned", False):
        return
    orig = hw_specs.get_activation_tables

    def patched(arch):
        tabs = dict(orig(arch))
        keep = "natural_log_exp_and_others"
        for name in list(tabs):
            if name == keep:
                continue
            fns = tabs[name]
            if any(str(f).endswith((".Exp", ".Ln")) for f in fns):
                tabs[name] = type(fns)()
        return tabs

    bacc_mod.get_activation_tables = patched
    bacc_mod._act_tables_pinned = True


def _kernel_body(ctx, tc, out_d, xe_d, xo_d, wkq_d, wv_d, denk_d, kmat_d,
                 woutT_d):
    nc = tc.nc

    consts = ctx.enter_context(tc.tile_pool(name="consts", bufs=1))
    planes = ctx.enter_context(tc.tile_pool(name="planes", bufs=1))
    xpool = ctx.enter_context(tc.tile_pool(name="xpool", bufs=1))
    small = ctx.enter_context(tc.tile_pool(name="small", bufs=2))
    rcpool = ctx.enter_context(tc.tile_pool(name="rcpool", bufs=2))
    prod_pool = ctx.enter_context(tc.tile_pool(name="prod", bufs=4))
    opre_pool = ctx.enter_context(tc.tile_pool(name="opre", bufs=2))
    outs_pool = ctx.enter_context(tc.tile_pool(name="outs", bufs=4))

    ps = ctx.enter_context(tc.tile_pool(name="ps", bufs=2, space="PSUM"))

    # ---- PE warm-up fodder: zero tile, no external deps ----
    wz = consts.tile([128, 512], BF16)
    nc.gpsimd.memset(wz, 0.0)
    for i in range(14):
        warm_ps = ps.tile([128, 512], F32, tag="mm", bufs=2, name="warm_ps")
        nc.tensor.matmul(warm_ps, wz[:, 0:128], wz, start=True, stop=True)

    # ---- constants into SBUF (plain DMAs, host-prepared layouts) ----
    wkq_sb = consts.tile([C, G], BF16)
    nc.sync.dma_start(out=wkq_sb, in_=wkq_d)
    wv_sb = consts.tile([C, CS], BF16)
    nc.sync.dma_start(out=wv_sb, in_=wv_d)
    denk_sb = consts.tile([G, 9, G], BF16)
    nc.sync.dma_start(out=denk_sb, in_=denk_d)
    kmat_sb = consts.tile([3 * G, 3, 2, 128], BF16)
    nc.sync.dma_start(out=kmat_sb, in_=kmat_d)
    woutT_sb = consts.tile([128, 2, CS], BF16)
    nc.sync.dma_start(out=woutT_sb, in_=woutT_d)

    # ---- x into SBUF, split even/odd row chunks for earlier compute ----
    x_sb = {}
    for b in range(BPC):
        x_sb[(b, 0)] = xpool.tile([C, H // 2, W], BF16, name=f"xe{b}")
        nc.sync.dma_start(out=x_sb[(b, 0)], in_=xe_d[b])
        x_sb[(b, 1)] = xpool.tile([C, H // 2, W], BF16, name=f"xo{b}")
        nc.sync.dma_start(out=x_sb[(b, 1)], in_=xo_d[b])

    # ---- persistent parity planes (zero pads set once) ----
    cost_pl = [planes.tile([G, PROWS, CCOLS], BF16, tag=f"cost{b}",
                           name=f"cost_pl{b}") for b in range(BPC)]
    v_pl = [[planes.tile([128, PROWS, VCOLS], BF16, tag=f"v{b}_{chn}",
                         name=f"v_pl{b}_{chn}") for chn in range(2)]
            for b in range(BPC)]
    for pl in cost_pl:
        nc.gpsimd.memset(pl[:, 28, :], 0.0)    # row -1
        nc.gpsimd.memset(pl[:, :, 0], 0.0)     # col -1 (O1 slot 0)
    for b in range(BPC):
        for chn in range(2):
            pl = v_pl[b][chn]
            nc.gpsimd.memset(pl[:, 28, :], 0.0)
            nc.gpsimd.memset(pl[:, :, 0], 0.0)

    def cview(b, di, dj):
        r0, c0 = ROW0[di], CCOL0[dj]
        return cost_pl[b][:, r0:r0 + 28, c0:c0 + 28]

    def vview(b, chn, di, dj):
        r0, c0 = ROW0[di], VCOL0[dj]
        return v_pl[b][chn][:, r0:r0 + 28, c0:c0 + 28]

    def dst_rows(rt):
        # row tile rt covers permuted rows 7rt..7rt+6; even tiles (rt<4) map
        # to plane rows 7rt.., odd tiles to plane rows 29+7(rt-4)..
        return 7 * rt if rt < 4 else 29 + 7 * (rt - 4)

    # ---- phase A1: qk matmuls + exp into cost planes ----
    def emit_a1(b):
        for rt in range(N_RT):
            half, idx = (0, rt) if rt < 4 else (1, rt - 4)
            qk_ps = ps.tile([G, RT, W], F32, tag="gam", bufs=3, name="qk_ps")
            nc.tensor.matmul(qk_ps, wkq_sb,
                             x_sb[(b, half)][:, idx * RT:(idx + 1) * RT, :],
                             start=True, stop=True)
            rd = dst_rows(rt)
            pl = cost_pl[b]
            # src cols: [0:28] odd (1,3..55), [28:56] even (0,2..54)
            nc.scalar.activation(out=pl[:, rd:rd + RT, 1:28],
                                 in_=qk_ps[:, :, 0:27],
                                 func=mybir.ActivationFunctionType.Exp)
            nc.scalar.activation(out=pl[:, rd:rd + RT, 28:56],
                                 in_=qk_ps[:, :, 28:56],
                                 func=mybir.ActivationFunctionType.Exp)
            nc.scalar.activation(out=pl[:, rd:rd + RT, 58:86],
                                 in_=qk_ps[:, :, 0:28],
                                 func=mybir.ActivationFunctionType.Exp)

    # ---- phase B: den conv as diagonal matmuls on PE + r = exp(-ln(den)) ----
    r_sb = {}

    def emit_den(b):
        den_ps = ps.tile([G, 2, 512], F32, tag="gam", bufs=3, name="den_ps")
        for s in range(N_STRIPS):
            dv = den_ps[:, s, :RPS * WO].rearrange("p (a c) -> p a c", a=RPS)
            for t, (di, dj) in enumerate(TAPS):
                nc.tensor.matmul(
                    dv, denk_sb[:, t, :],
                    cview(b, di, dj)[:, s * RPS:(s + 1) * RPS, :],
                    start=(t == 0), stop=(t == 8))
        lden = small.tile([G, 2, RPS, WO], F32, tag="lden", name="lden")
        dfull = den_ps[:, :, :RPS * WO].rearrange("p s (a c) -> p s a c",
                                                  a=RPS)
        nc.scalar.activation(out=lden, in_=dfull,
                             func=mybir.ActivationFunctionType.Ln)
        r_sb[b] = small.tile([G, 2, RPS, WO], BF16, tag="rr", name="rr")
        nc.scalar.activation(out=r_sb[b], in_=lden, scale=-1.0,
                             func=mybir.ActivationFunctionType.Exp)

    # interleave so the PE has den work while the other batch's exps drain
    emit_a1(0)
    emit_den(0)
    emit_a1(1)
    emit_den(1)

    # ---- phase A2: v matmuls -> v planes ----
    for b in range(BPC):
        for chn in range(2):
            for rt in range(N_RT):
                half, idx = (0, rt) if rt < 4 else (1, rt - 4)
                v_ps = ps.tile([128, RT, W], F32, tag="mm", bufs=2,
                               name="v_ps")
                nc.tensor.matmul(v_ps, wv_sb[:, chn * 128:(chn + 1) * 128],
                                 x_sb[(b, half)][:, idx * RT:(idx + 1) * RT, :],
                                 start=True, stop=True)
                rd = dst_rows(rt)
                # one drain: dst cols [1:57] = [O slots 1:29 | E 29:57],
                # src [0:28]=odd, [28:56]=even
                dst = v_pl[b][chn][:, rd:rd + RT, 1:57].rearrange(
                    "p r (g c) -> p r g c", g=2)
                src = v_ps.rearrange("p r (g c) -> p r g c", g=2)
                nc.scalar.copy(out=dst, in_=src)

    # ---- phase C: rc[t] = cost_t * r (DVE, packed 4x mode) ----
    rc_st = {}
    for b in range(BPC):
        rc_st[b] = [rcpool.tile([3 * G, 2, RPS, WO], BF16,
                                tag=f"rc{grp}", name=f"rc_st{grp}")
                    for grp in range(3)]
        for t, (di, dj) in enumerate(TAPS):
            grp, tau = divmod(t, 3)
            cvs = cview(b, di, dj).rearrange("p (s a) c -> p s a c", s=2)
            nc.vector.tensor_mul(rc_st[b][grp][tau * G:(tau + 1) * G],
                                 cvs, r_sb[b])

    # ---- phases D+E per batch: packed gamma matmuls, products on DVE,
    #      tap-sum tree on GpSimd ----
    opre_sb = {}
    gsums = {}
    for grp in range(3):
        for b in range(BPC):
            for chn in range(2):
                gams = [ps.tile([128, 2, 512], F32, tag="gam", bufs=3,
                                name=f"gam_ps{tau}") for tau in range(3)]
                for s in range(N_STRIPS):
                    for tau in range(3):
                        gv = gams[tau][:, s, :RPS * WO].rearrange(
                            "p (a c) -> p a c", a=RPS)
                        nc.tensor.matmul(
                            gv, kmat_sb[tau * G:(tau + 1) * G, grp, chn, :],
                            rc_st[b][grp][tau * G:(tau + 1) * G, s],
                            start=True, stop=True,
                            tile_position=(tau * G, 0))
                ps_taps = []
                for tau in range(3):
                    t = grp * 3 + tau
                    di, dj = TAPS[t]
                    p_sb = prod_pool.tile([128, 2, RPS, WO], BF16,
                                          tag="p", bufs=9, name=f"p{tau}")
                    gfull = gams[tau][:, :, :RPS * WO].rearrange(
                        "p s (a c) -> p s a c", a=RPS)
                    vv = vview(b, chn, di, dj).rearrange(
                        "p (s a) c -> p s a c", s=2)
                    nc.vector.tensor_mul(p_sb, gfull, vv)
                    ps_taps.append(p_sb)
                gs = prod_pool.tile([128, 2 * RPS * WO], BF16,
                                    tag=f"gs{grp}_{b}_{chn}", bufs=1,
                                    name=f"gs{grp}{b}{chn}")
                flat = [p.rearrange("p s a c -> p (s a c)") for p in ps_taps]
                nc.vector.tensor_add(gs, flat[0], flat[1])
                nc.gpsimd.tensor_add(gs, gs, flat[2])
                gsums[(grp, b, chn)] = gs
    for b in range(BPC):
        for chn in range(2):
            o_sb = opre_pool.tile([128, 2, RPS, WO], BF16,
                                  tag=f"opre{chn}", name=f"opre{chn}")
            of = o_sb.rearrange("p s a c -> p (s a c)")
            nc.vector.tensor_add(of, gsums[(0, b, chn)], gsums[(1, b, chn)])
            nc.vector.tensor_add(of, of, gsums[(2, b, chn)])
            for s in range(N_STRIPS):
                opre_sb[(b, chn, s)] = o_sb[:, s]

        for mo in range(2):
            for s in range(N_STRIPS):
                out_ps = ps.tile([128, RPS, WO], F32, tag="mm",
                                 bufs=2, name="out_ps")
                for kc in range(2):
                    nc.tensor.matmul(out_ps,
                                     woutT_sb[:, kc, mo * 128:(mo + 1) * 128],
                                     opre_sb[(b, kc, s)],
                                     start=(kc == 0), stop=(kc == 1))
                o_final = outs_pool.tile([128, RPS, WO], F32)
                nc.scalar.copy(out=o_final, in_=out_ps)
                nc.sync.dma_start(
                    out=out_d[b, mo * 128:(mo + 1) * 128,
                              s * RPS:(s + 1) * RPS, :],
                    in_=o_final)


def _install_ntff_shim():
    """bass_utils expects antenv.axon_hooks (absent in this checkout); shim it
    with the ctypes NTFF hook from trn_agent_boot so trace=True works."""
    import sys
    import types
    try:
        from antenv.axon_hooks import get_axon_ntff_profile_hook  # noqa: F401
        return
    except ImportError:
        pass
    try:
        from trn_agent_boot.trn_boot import _ntff_profile_via_ctypes
        hook = _ntff_profile_via_ctypes("/opt/axon/libaxon_pjrt.so")
    except Exception:
        hook = None
    mod = types.ModuleType("antenv.axon_hooks")
    mod._hook = hook
    mod.get_axon_ntff_profile_hook = lambda: mod._hook
    mod.set_axon_ntff_profile_hook = lambda h: setattr(mod, "_hook", h)
    sys.modules["antenv.axon_hooks"] = mod


def _get_program():
    if "nc" not in _BUILD_CACHE:
        _BUILD_CACHE["nc"] = _build_program()
    return _BUILD_CACHE["nc"]


# host-side pixel permutation: rows even-first, cols odd-first
_ROW_PERM = np.concatenate([np.arange(0, H, 2), np.arange(1, H, 2)])
_COL_PERM = np.concatenate([np.arange(1, W, 2), np.arange(0, W, 2)])


def kernel(x, Wk, Wv, Wout, q_param, attn_scale, rpb_table):
    import ml_dtypes
    x = np.asarray(x, dtype=np.float32)
    xp = x[:, :, _ROW_PERM][:, :, :, _COL_PERM].astype(ml_dtypes.bfloat16)
    wts = _host_weights(np.asarray(Wk), np.asarray(Wv), np.asarray(Wout),
                        np.asarray(q_param), np.asarray(attn_scale),
                        np.asarray(rpb_table))
    nc = _get_program()

    in_maps = []
    for c in range(NCORES):
        m = dict(wts)
        for b in range(BPC):
            xb = xp[c * BPC + b]
            m[f"xe{b}"] = np.ascontiguousarray(xb[:, :H // 2])
            m[f"xo{b}"] = np.ascontiguousarray(xb[:, H // 2:])
        in_maps.append(m)

    trace = bool(int(os.environ.get("KERNEL_TRACE", "0")))
    if trace:
        _install_ntff_shim()
    res = run_bass_kernel_spmd(nc, in_maps, core_ids=list(range(NCORES)),
                               trace=trace)
    _BUILD_CACHE["last_results"] = res

    out = np.empty((B, CS, HO, WO), np.float32)
    for c in range(NCORES):
        out[c * BPC:(c + 1) * BPC] = res.results[c]["out"]
    return out
